# revision 1
# baseline (speedup 1.0000x reference)
"""Trainium2 Bass kernel v3 for per-(batch,channel) circular autocorrelation:

    out = ifft2(|fft2(img - mean(img))|^2).real / (H*W)

Dense-DFT formulation (no FFT primitive on TRN2); per 256x256 image:
  s1: [U|V] = d^T [CoH|SiH]          4 fp32r matmuls, 258 cols (FD>=256 -> full rate)
  s2: [G1|G2] = Co[U|V], (-Si)V, SiU 12 bf16 matmuls, 1032 col-cycles/half
      (negated-Si constants instead of a DVE-prepared [-V|U] operand)
  P:  P = G1^2 + G2^2 (bf16), DC bin zeroed (== mean subtraction)
  s4a: Q = P^T [Co|Si] over x=0..128 ONLY (Qr is x-symmetric, Qi
       x-antisymmetric), rows j=128..1; the DC (j=0) cosine row Qr0 is
       rank-1 accumulated into the Nyquist Qi slot (exactly 0 by symmetry).
       Full-x [Qr|Qi] assembled by reversed-stride mirror copies.
  s4b: out[y,:] = cw'^T Qr + sw'^T Qi'   4 bf16 matmuls, 256 cols
       cw' rows j=128..1 (w_j in {1,2,...,2}); sw' row 0 := s absorbs the
       DC term (y-independent) -- both special rows cost zero extra matmuls.

~4.9k PE col-cycles/image (~2.0us); 4-stage software pipeline at stage
distance 2. Squares on ACT, PSUM->SBUF copies+mirrors on DVE/ACT, memset on
GpSimd, DMAs on SP. PSUM: 8 banks = p1(2) p2(2) p4(1)x2bufs p5(1)x2bufs.

Sharding: pure data parallel, 8 batches per core (64 images of 64x8 b/c).
"""

import numpy as np

N = 256
J = N // 2 + 1  # 129
B, H, W, C = 64, 256, 256, 8
N_CORES = 8
IMGS_PER_CORE = (B // N_CORES) * C  # 64


def _make_consts():
    import ml_dtypes

    a = np.arange(N, dtype=np.float64)
    # half-spectrum stored j-REVERSED: column c <-> j = 128 - c, so that the
    # special rows land on partition 0 downstream (matmul output base
    # partition must be 0/32/64).
    j = np.arange(128, -1, -1, dtype=np.float64)  # 128..0
    ang = 2.0 * np.pi / N

    CoH = np.cos(ang * np.outer(a, j))  # [256, 129]
    SiH = np.sin(ang * np.outer(a, j))
    cosih = np.concatenate([CoH, SiH], axis=1).astype(np.float32)  # [256, 258]

    Co = np.cos(ang * np.outer(a, a))
    Si = np.sin(ang * np.outer(a, a))
    bf = ml_dtypes.bfloat16
    cosi_bf = np.concatenate([Co, Si], axis=1).astype(bf)  # [256, 512]
    nsi_bf = (-Si).astype(bf)  # [256, 256]
    # s4a rhs: only x = 0..128 needed (Qr symmetric / Qi antisymmetric in x)
    csh_bf = np.concatenate([Co[:, 0:129], Si[:, 0:129]], axis=1).astype(bf)

    s = 1.0 / float(N) ** 4
    jm = np.arange(128, 0, -1, dtype=np.float64)  # j = 128..1 (row m <-> 128-m)
    w = np.full(128, 2.0)
    w[0] = 1.0  # j=128 (Nyquist) counted once
    cw = (s * w[:, None] * np.cos(ang * np.outer(jm, a))).astype(bf)  # [128, 256]
    swneg = -s * w[:, None] * np.sin(ang * np.outer(jm, a))
    swneg[0, :] = s  # DC row (j=0, w=1, cos(0)=1): y-independent
    swneg = swneg.astype(bf)

    return dict(
        cosih=cosih,
        cosi_bf=np.ascontiguousarray(cosi_bf),
        nsi_bf=np.ascontiguousarray(nsi_bf),
        csh_bf=np.ascontiguousarray(csh_bf),
        cw=np.ascontiguousarray(cw),
        swneg=np.ascontiguousarray(swneg),
    )


def build_program(n_imgs=IMGS_PER_CORE, n_cores=N_CORES, n_iter=1):
    """Build the Bass/Tile program. Returns nc.

    n_iter > 1 wraps the whole image pipeline in a For_i hardware loop that
    repeats it n_iter times (same inputs/outputs each pass) — used only by
    the timing harness to measure per-pass device time without compiling a
    huge unrolled program.
    """
    from contextlib import ExitStack

    import concourse.bacc as bacc
    import concourse.tile as tile
    from concourse import mybir

    f32 = mybir.dt.float32
    f32r = mybir.dt.float32r
    bf16 = mybir.dt.bfloat16

    nc = bacc.Bacc(
        "TRN2",
        target_bir_lowering=False,
        debug=False,
        num_devices=n_cores,
    )

    x_d = nc.dram_tensor("x", [n_imgs, N, N], f32, kind="ExternalInput").ap()
    cosih_d = nc.dram_tensor("cosih", [N, 258], f32, kind="ExternalInput").ap()
    cosi_d = nc.dram_tensor("cosi_bf", [N, 512], bf16, kind="ExternalInput").ap()
    nsi_d = nc.dram_tensor("nsi_bf", [N, 256], bf16, kind="ExternalInput").ap()
    csh_d = nc.dram_tensor("csh_bf", [N, 258], bf16, kind="ExternalInput").ap()
    cw_d = nc.dram_tensor("cw", [128, N], bf16, kind="ExternalInput").ap()
    swneg_d = nc.dram_tensor("swneg", [128, N], bf16, kind="ExternalInput").ap()
    out_d = nc.dram_tensor("out", [n_imgs, N, N], f32, kind="ExternalOutput").ap()

    with tile.TileContext(nc) as tc, ExitStack() as ctx:
        singles = ctx.enter_context(tc.tile_pool(name="singles", bufs=1))
        dpool = ctx.enter_context(tc.tile_pool(name="dpool", bufs=4))
        uvpool = ctx.enter_context(tc.tile_pool(name="uvpool", bufs=4))
        ppool = ctx.enter_context(tc.tile_pool(name="ppool", bufs=4))
        tpool = ctx.enter_context(tc.tile_pool(name="tpool", bufs=6))
        qpool = ctx.enter_context(tc.tile_pool(name="qpool", bufs=4))
        opool = ctx.enter_context(tc.tile_pool(name="opool", bufs=6))
        ps1 = ctx.enter_context(tc.tile_pool(name="ps1", bufs=1, space="PSUM"))
        ps2 = ctx.enter_context(tc.tile_pool(name="ps2", bufs=1, space="PSUM"))
        ps4 = ctx.enter_context(tc.tile_pool(name="ps4", bufs=2, space="PSUM"))
        ps5 = ctx.enter_context(tc.tile_pool(name="ps5", bufs=2, space="PSUM"))

        # --- constants into SBUF ---
        cosih = [singles.tile([128, 258], f32r, tag=f"cosih{h}", name=f"cosih{h}") for h in range(2)]
        cosib = [singles.tile([128, 512], bf16, tag=f"cosib{h}", name=f"cosib{h}") for h in range(2)]
        nsib = [singles.tile([128, 256], bf16, tag=f"nsib{h}", name=f"nsib{h}") for h in range(2)]
        cshb = [singles.tile([128, 258], bf16, tag=f"cshb{h}", name=f"cshb{h}") for h in range(2)]
        for h in range(2):
            sl = slice(128 * h, 128 * (h + 1))
            nc.gpsimd.dma_start(out=cosih[h], in_=cosih_d[sl, :].bitcast(f32r))
            nc.gpsimd.dma_start(out=cosib[h], in_=cosi_d[sl, :])
            nc.gpsimd.dma_start(out=nsib[h], in_=nsi_d[sl, :])
            nc.gpsimd.dma_start(out=cshb[h], in_=csh_d[sl, :])
        cw = singles.tile([128, N], bf16, tag="cw", name="cw")
        swneg = singles.tile([128, N], bf16, tag="swneg", name="swneg")
        nc.gpsimd.dma_start(out=cw, in_=cw_d)
        nc.gpsimd.dma_start(out=swneg, in_=swneg_d)

        mm = nc.tensor.matmul

        st = {}

        def stageA(i):
            # load + s1: [U|V] = d^T [CoH|SiH]  (fp32r, FD=258)
            d = dpool.tile([128, 2, N], f32r, tag="d", name="d")
            nc.sync.dma_start(
                out=d,
                in_=x_d[i].rearrange("(h p) c -> p h c", h=2).bitcast(f32r),
            )
            p1 = ps1.tile([128, 2, 512], f32, tag="s1", name="s1")
            for xh in range(2):
                xs = slice(128 * xh, 128 * (xh + 1))
                mm(p1[:, xh, 0:258], d[:, 0, xs], cosih[0], start=True, stop=False)
                mm(p1[:, xh, 0:258], d[:, 1, xs], cosih[1], start=False, stop=True)
            uv = uvpool.tile([128, 2, 258], bf16, tag="uv", name="uv")
            # split the PSUM->SBUF copy across ACT/DVE to balance engine load
            nc.scalar.activation(out=uv[:, 0, :], in_=p1[:, 0, 0:258],
                                 func=mybir.ActivationFunctionType.Copy)
            nc.vector.tensor_copy(out=uv[:, 1, :], in_=p1[:, 1, 0:258])
            st[i] = {"uv": uv}

        def stageB(i):
            # s2: G1 = Co U - Si V ; G2 = Si U + Co V  (bf16, FD=258), then
            # P = G1^2 + G2^2 with DC bin zeroed
            uv = st[i]["uv"]
            p2 = ps2.tile([128, 2, 512], f32, tag="s2", name="s2")
            for kt in range(2):
                ks = slice(128 * kt, 128 * (kt + 1))
                ss = slice(256 + 128 * kt, 256 + 128 * (kt + 1))
                out = p2[:, kt, 0:258]
                o1 = p2[:, kt, 0:129]    # G1 = CoU - SiV
                o2 = p2[:, kt, 129:258]  # G2 = CoV + SiU
                # Co x [U|V] -> [CoU|CoV]; then (-Si)xV into G1, Si x U into
                # G2 -- same PE cycles as the uvs variant, no DVE prep needed
                mm(out, cosib[0][:, ks], uv[:, 0, :], start=True, stop=False)
                mm(o1, nsib[0][:, ks], uv[:, 0, 129:258], start=False, stop=False)
                mm(o1, nsib[1][:, ks], uv[:, 1, 129:258], start=False, stop=False)
                mm(o2, cosib[0][:, ss], uv[:, 0, 0:129], start=False, stop=False)
                mm(o2, cosib[1][:, ss], uv[:, 1, 0:129], start=False, stop=False)
                mm(out, cosib[1][:, ks], uv[:, 1, :], start=False, stop=True)
            tsq = tpool.tile([128, 2, 258], bf16, tag="tsq", name="tsq")
            nc.scalar.activation(out=tsq, in_=p2[:, :, 0:258],
                                 func=mybir.ActivationFunctionType.Square)
            P = ppool.tile([128, 2, 129], bf16, tag="P", name="P")
            # SBUF-only elementwise add on the otherwise-idle GpSimd engine
            nc.gpsimd.tensor_add(P, tsq[:, :, 0:129], tsq[:, :, 129:258])
            nc.gpsimd.memset(P[0:1, 0, 128:129], 0.0)  # DC bin (k=0, j=0)
            st[i]["P"] = P

        def stageC(i):
            # s4a over x = 0..128 only: Qr is x-symmetric, Qi x-antisymmetric.
            # p4[m, 0:129] = Qr, p4[m, 129:258] = Qi, rows j = 128-m (m=0 is
            # the Nyquist row, whose Qi is ~0 by symmetry); the DC (j=0)
            # cosine row Qr0 is rank-1 accumulated into that slot.
            P = st[i]["P"]
            p4 = ps4.tile([128, 258], f32, tag="s4a", name="s4a")
            mm(p4, P[:, 0, 0:128], cshb[0], start=True, stop=False)
            mm(p4[0:1, 129:258], P[:, 0, 128:129], cshb[0][:, 0:129],
               start=False, stop=False)
            mm(p4[0:1, 129:258], P[:, 1, 128:129], cshb[1][:, 0:129],
               start=False, stop=False)
            mm(p4, P[:, 1, 0:128], cshb[1], start=False, stop=True)
            # assemble full-x [Qr | Qi] in SBUF: direct block on ACT, mirror
            # halves via reversed-stride copies (x>=129 <- mirror of 127..1)
            qrqi = qpool.tile([128, 2, 256], bf16, tag="qrqi", name="qrqi")
            nc.scalar.activation(out=qrqi[:, :, 0:129],
                                 in_=p4.rearrange("p (b c) -> p b c", b=2),
                                 func=mybir.ActivationFunctionType.Copy)
            nc.vector.tensor_copy(out=qrqi[:, 0, 129:256], in_=p4[:, 127:0:-1])
            nc.vector.tensor_scalar_mul(qrqi[:, 1, 129:256],
                                        p4[:, 256:129:-1], -1.0)
            # row 0 (Nyquist Qi ~0 + absorbed DC Qr0) mirrors POSITIVELY
            nc.vector.tensor_copy(out=qrqi[0:1, 1, 129:256],
                                  in_=p4[0:1, 256:129:-1])
            st[i]["qrqi"] = qrqi

        def stageD(i):
            # s4b: y rows 0..127 (cols 0:256) and 128..255 (cols 256:512);
            # swneg's last row carries the DC term (see _make_consts)
            qrqi = st[i]["qrqi"]
            qr = qrqi[:, 0, :]
            qi = qrqi[:, 1, :]
            p5 = ps5.tile([128, 512], f32, tag="s4b", name="s4b")
            top = p5[:, 0:256]
            bot = p5[:, 256:512]
            mm(top, cw[:, 0:128], qr, start=True, stop=False)
            mm(top, swneg[:, 0:128], qi, start=False, stop=True)
            mm(bot, cw[:, 128:256], qr, start=True, stop=False)
            mm(bot, swneg[:, 128:256], qi, start=False, stop=True)
            o = opool.tile([128, 512], f32, tag="o", name="o")
            nc.vector.tensor_copy(out=o, in_=p5)
            nc.sync.dma_start(
                out=out_d[i].rearrange("(h p) c -> p h c", h=2),
                in_=o.rearrange("p (h c) -> p h c", h=2),
            )
            del st[i]

        # software pipeline, stage distance 2: producers get a full extra
        # tick of slack before their consumer stage runs. Deepest stage
        # first, so no engine's stream blocks on a same-image downstream dep.
        def pipeline():
            for t in range(n_imgs + 6):
                if 0 <= t - 6 < n_imgs:
                    stageD(t - 6)
                if 0 <= t - 4 < n_imgs:
                    stageC(t - 4)
                if 0 <= t - 2 < n_imgs:
                    stageB(t - 2)
                if t < n_imgs:
                    stageA(t)

        if n_iter == 1:
            pipeline()
        else:
            with tc.For_i(0, n_iter, 1):
                pipeline()

    nc.compile()
    return nc


_CACHED = {}


def _get_program(n_imgs, n_cores):
    key = (n_imgs, n_cores)
    if key not in _CACHED:
        _CACHED[key] = build_program(n_imgs, n_cores)
    return _CACHED[key]


def kernel(inputs: np.ndarray) -> np.ndarray:
    """inputs: [64, 256, 256, 8] float32 -> output same shape."""
    from concourse.bass_utils import run_bass_kernel_spmd

    inputs = np.asarray(inputs, dtype=np.float32)
    assert inputs.shape == (B, H, W, C)

    consts = _make_consts()
    nc = _get_program(IMGS_PER_CORE, N_CORES)

    bpc = B // N_CORES  # batches per core
    in_maps = []
    for core in range(N_CORES):
        shard = inputs[core * bpc:(core + 1) * bpc]  # [8, 256, 256, 8]
        shard = np.ascontiguousarray(shard.transpose(0, 3, 1, 2)).reshape(
            IMGS_PER_CORE, H, W
        )
        m = {"x": shard}
        m.update(consts)
        in_maps.append(m)

    res = run_bass_kernel_spmd(nc, in_maps, core_ids=list(range(N_CORES)))

    out = np.empty((B, H, W, C), dtype=np.float32)
    for core in range(N_CORES):
        o = res.results[core]["out"].reshape(bpc, C, H, W)
        out[core * bpc:(core + 1) * bpc] = o.transpose(0, 2, 3, 1)
    return out


if __name__ == "__main__":
    rng = np.random.default_rng(0)
    x = rng.standard_normal((B, H, W, C)).astype(np.float32)
    y = kernel(x)
    print("kernel output:", y.shape, y.dtype)

